# revision 6
# baseline (speedup 1.0000x reference)
"""Trainium2 Bass kernel for nn_BlockLinear forward — hybrid fp16/fp8 PE version.

Computes y[b, o] = sum_k exp(log_weight[o, k]) * x[b, o*K + k]
for x [16384, 8192] fp32, log_weight [1024, 8] fp32.

Data-parallel over batch across 8 NeuronCores (2048 rows each).  Builds
on the fp16 PE kernel (112us), cutting DMA bytes further with weight-
aware mixed precision: for each output o, the 2 features with the
largest w = exp(log_weight) stay fp16 ("hot"), the remaining 6 go fp8
e4m3 ("cold").  Quantization error is dominated by w-amplified terms,
so routing only the top-2 weights per output through fp16 gives rel err
6.7e-3 (measured on the fixed-seed inputs; tolerance 2e-2) while the
x stream shrinks 33.5 MB -> 21 MB/core (+4.2 MB y out).

The host permutes features into per-output-sorted order, which makes
EVERY stationary diagonal: chunk c = 8t + j of group t holds feature
rank j of output o = 128t + p at row p, so S_c = diag(w-rank-j).  The
device builds all 64 [128,128] stationary blocks from a 16KB weight
table with one tensor_scalar_mul each against an uploaded identity
(per-partition scalar broadcast; fp16 out for hot chunks, fp8 for cold).

Per group and batch-quarter b: 2 hot fp16 matmuls accumulate in a hot
PSUM bank, 6 cold fp8 matmuls in a cold bank (groups never mix dtypes
in one accumulation bank — mixed groups wedge the device, as do
mixed-K partial accumulations).  The hot bank evacuates early via ACT
copy to an fp32 staging tile (freeing hot banks mid-group), then DVE
fuses (cold + staged hot) -> fp16 with one scalar_tensor_tensor per
bank.  Stores batch per group into one [128, 2048] DMA (4KB
descriptors; 1KB-descriptor stores are desc-gen-bound at ~6ns/desc).

Streams: hot x [2048, 2048] f16 + cold x [6144, 2048] f8 + y out
[1024, 2048] f16 = 25.2 MB/core vs 37.8 before.  x loads ride Sync
HWDGE; tables + y stores ride ScalarE HWDGE.  HWDGE queues cannot
start before ~8.7us (NEFF preamble).  PE: 256 matmuls x 512 cols +
self-loaded stationaries ~= 68us, roughly matching the stream.
"""

import numpy as np

B = 16384
IN_F = 8192
OUT_F = 1024
K = 8
N_CORES = 8
P = 128
N_HOT = 1  # hot (fp16) feature ranks per output; rest are fp8

_CACHE = {}


def _build(b_shard, in_f, out_f, n_cores, x_bufs=6, c_bufs=18, y_bufs=3, nb=4):
    """Build + compile the per-core Bass module (SPMD across n_cores)."""
    from concourse import bacc, tile, mybir

    f16 = mybir.dt.float16
    f8 = mybir.dt.float8e4
    f32 = mybir.dt.float32
    n_groups = out_f // P  # 8 output groups of 128
    n_cold = 8 - N_HOT
    bw = b_shard // nb  # batch columns per PSUM bank (512 fp32 = 1 bank)

    nc = bacc.Bacc(
        "TRN2",
        target_bir_lowering=False,
        debug=False,
        enable_asserts=True,
        num_devices=n_cores,
    )
    xh_d = nc.dram_tensor("xh", [n_groups * N_HOT * P, b_shard], f16, kind="ExternalInput")
    xc_d = nc.dram_tensor("xc", [n_groups * n_cold * P, b_shard], f8, kind="ExternalInput")
    v_d = nc.dram_tensor("v", [P, 8 * n_groups], f32, kind="ExternalInput")
    id_d = nc.dram_tensor("idm", [P, P], f16, kind="ExternalInput")
    y_d = nc.dram_tensor("y", [out_f, b_shard], f16, kind="ExternalOutput")

    with tile.TileContext(nc) as tc:
        with (
            tc.tile_pool(name="consts", bufs=1) as cpool,
            tc.tile_pool(name="xh", bufs=x_bufs) as hpool,
            tc.tile_pool(name="xc", bufs=c_bufs) as cxpool,
            tc.tile_pool(name="ys", bufs=y_bufs) as ypool,
            tc.tile_pool(name="yh", bufs=2) as spool,
            tc.tile_pool(name="ps", bufs=4, space="PSUM") as ppool,
        ):
            ident = cpool.tile([P, P], f16, tag="id")
            vt = cpool.tile([P, 8 * n_groups], f32, tag="v")
            st16 = cpool.tile([P, n_groups * N_HOT * P], f16, tag="s16")
            st8 = cpool.tile([P, n_groups * n_cold * P], f8, tag="s8")
            nc.scalar.dma_start(out=ident[:], in_=id_d[:])
            nc.scalar.dma_start(out=vt[:], in_=v_d[:])
            for t in range(n_groups):
                last = t == n_groups - 1
                # Group t's diagonal stationaries: S = ident * v[:, c]
                # broadcast per partition (v[p, 8t+j] = rank-j weight of
                # output 128t+p).  Emitted INSIDE the loop so the DVE FIFO
                # interleaves builds with the PSUM-freeing fuse ops — built
                # up front, every fuse queues behind ~15us of builds and the
                # PE stalls on PSUM recycling.
                from concourse import mybir as mb

                id_h = ident[:].rearrange("p (x m) -> p x m", x=1).broadcast_to([P, N_HOT, P])
                v_h = (
                    vt[:, 8 * t : 8 * t + N_HOT]
                    .rearrange("p (j x) -> p j x", x=1)
                    .broadcast_to([P, N_HOT, P])
                )
                nc.vector.scalar_tensor_tensor(
                    st16[:, t * N_HOT * P : (t + 1) * N_HOT * P].rearrange(
                        "p (j m) -> p j m", j=N_HOT
                    ),
                    id_h, 1.0, v_h, mb.AluOpType.mult, mb.AluOpType.mult,
                )
                id_c = ident[:].rearrange("p (x m) -> p x m", x=1).broadcast_to([P, n_cold, P])
                v_c = (
                    vt[:, 8 * t + N_HOT : 8 * t + 8]
                    .rearrange("p (j x) -> p j x", x=1)
                    .broadcast_to([P, n_cold, P])
                )
                nc.vector.scalar_tensor_tensor(
                    st8[:, t * n_cold * P : (t + 1) * n_cold * P].rearrange(
                        "p (j m) -> p j m", j=n_cold
                    ),
                    id_c, 1.0, v_c, mb.AluOpType.mult, mb.AluOpType.mult,
                )
                hbs = [
                    ppool.tile([P, bw], f32, tag="ph", name=f"ph_{t}_{b}")
                    for b in range(nb)
                ]
                cbs = [
                    ppool.tile([P, bw], f32, tag="pc", name=f"pc_{t}_{b}")
                    for b in range(nb)
                ]
                # loads in consumption order: hot ranks 0..1, then cold 0..5
                hts, cts = [], []
                for j in range(N_HOT):
                    r = (t * N_HOT + j) * P
                    xt = hpool.tile([P, b_shard], f16, tag="xh", name=f"xh_{t}_{j}")
                    nc.sync.dma_start(out=xt[:], in_=xh_d[r : r + P, :])
                    hts.append(xt)
                for j in range(n_cold):
                    r = (t * n_cold + j) * P
                    xt = cxpool.tile([P, b_shard], f8, tag="xc", name=f"xc_{t}_{j}")
                    nc.sync.dma_start(out=xt[:], in_=xc_d[r : r + P, :])
                    cts.append(xt)
                # hot accumulation (fp16 x fp16), 2 matmuls per bank
                for j in range(N_HOT):
                    hc = t * N_HOT + j
                    for b in range(nb):
                        mm = nc.tensor.matmul(
                            hbs[b][:],
                            lhsT=st16[:, hc * P : (hc + 1) * P],
                            rhs=hts[j][:, b * bw : (b + 1) * bw],
                            start=(j == 0),
                            stop=(j == N_HOT - 1),
                        )
                        if b > 0:
                            # stationary already in the PE array from b=0:
                            # skip the redundant 128-cycle weight reload
                            mm.ins.ldweights = False
                # evacuate hot banks early to fp32 staging (frees them for
                # the next group while cold matmuls still run)
                ysb = spool.tile([P, b_shard], f32, tag="yh", name=f"yh_{t}")
                for b in range(nb):
                    nc.scalar.copy(out=ysb[:, b * bw : (b + 1) * bw], in_=hbs[b][:])
                # cold accumulation (fp8 x fp8), 6 matmuls per bank
                for j in range(n_cold):
                    cc = t * n_cold + j
                    for b in range(nb):
                        mm = nc.tensor.matmul(
                            cbs[b][:],
                            lhsT=st8[:, cc * P : (cc + 1) * P],
                            rhs=cts[j][:, b * bw : (b + 1) * bw],
                            start=(j == 0),
                            stop=(j == n_cold - 1),
                        )
                        if b > 0:
                            mm.ins.ldweights = False
                yt = ypool.tile([P, b_shard], f16, tag="y", name=f"y_{t}")
                if last:
                    # tail: fuse straight from PSUM per bank (no staging
                    # copy), store each half as soon as its banks fuse
                    for b in range(nb):
                        nc.vector.scalar_tensor_tensor(
                            yt[:, b * bw : (b + 1) * bw],
                            cbs[b][:],
                            1.0,
                            ysb[:, b * bw : (b + 1) * bw],
                            mb.AluOpType.mult,
                            mb.AluOpType.add,
                        )
                        if b == 1:
                            nc.scalar.dma_start(
                                out=y_d[t * P : (t + 1) * P, 0 : 2 * bw],
                                in_=yt[:, 0 : 2 * bw],
                            )
                    nc.scalar.dma_start(
                        out=y_d[t * P : (t + 1) * P, 2 * bw :], in_=yt[:, 2 * bw :]
                    )
                else:
                    # free cold banks fast via ACT copies to fp32 staging;
                    # fuse cold + hot -> fp16 on DVE off the PSUM critical path
                    csb = spool.tile([P, b_shard], f32, tag="yc", name=f"yc_{t}")
                    for b in range(nb):
                        nc.scalar.copy(out=csb[:, b * bw : (b + 1) * bw], in_=cbs[b][:])
                    for b in range(nb):
                        nc.vector.scalar_tensor_tensor(
                            yt[:, b * bw : (b + 1) * bw],
                            csb[:, b * bw : (b + 1) * bw],
                            1.0,
                            ysb[:, b * bw : (b + 1) * bw],
                            mb.AluOpType.mult,
                            mb.AluOpType.add,
                        )
                    nc.scalar.dma_start(out=y_d[t * P : (t + 1) * P, :], in_=yt[:])
    nc.compile()
    return nc


def _prep_inputs(x, log_weight, b_shard):
    """Host-side: per-output weight sort, permutation gather, mixed casts."""
    from concourse import mybir

    f8np = mybir.dt.np(mybir.dt.float8e4)
    w = np.exp(np.asarray(log_weight, np.float64)).astype(np.float32)  # [1024, 8]
    ordk = np.argsort(-w, axis=1)  # [1024, 8] feature ranks per output
    o_all = np.arange(OUT_F)
    wsort = w[o_all[:, None], ordk]  # [1024, 8]
    # v[p, 8t+j] = wsort[128t+p, j]
    v = np.ascontiguousarray(
        wsort.reshape(8, P, 8).transpose(1, 0, 2).reshape(P, 64), dtype=np.float32
    )
    feat = (8 * o_all[:, None] + ordk).reshape(8, P, 8)  # [t, p, j]
    hot_idx = feat[:, :, :N_HOT].transpose(0, 2, 1).reshape(-1)  # [(t*2+j)*128+p]
    cold_idx = feat[:, :, N_HOT:].transpose(0, 2, 1).reshape(-1)
    ident = np.eye(P, dtype=np.float16)

    x16 = np.asarray(x, np.float32).astype(np.float16)
    in_maps = []
    for i in range(N_CORES):
        xt = np.ascontiguousarray(x16[i * b_shard : (i + 1) * b_shard].T)
        xh = np.ascontiguousarray(xt[hot_idx])
        xc = np.ascontiguousarray(xt[cold_idx]).astype(f8np)
        in_maps.append({"xh": xh, "xc": xc, "v": v, "idm": ident})
    return in_maps


def kernel(x, log_weight, _trace_dir=None):
    from concourse import bass_utils

    b_shard = B // N_CORES
    if "nc" not in _CACHE:
        _CACHE["nc"] = _build(b_shard, IN_F, OUT_F, N_CORES)
    nc = _CACHE["nc"]

    in_maps = _prep_inputs(x, log_weight, b_shard)
    kwargs = {}
    if _trace_dir is not None:
        kwargs = {"trace": True, "tmpdir": _trace_dir}
    res = bass_utils.run_bass_kernel_spmd(
        nc, in_maps, core_ids=list(range(N_CORES)), **kwargs
    )
    _CACHE["last_res"] = res
    y = np.empty((B, OUT_F), np.float32)
    for i in range(N_CORES):
        y[i * b_shard : (i + 1) * b_shard] = res.results[i]["y"].T.astype(np.float32)
    return y


# revision 7
# speedup vs baseline: 1.0140x; 1.0140x over previous
"""Trainium2 Bass kernel for nn_BlockLinear forward — hybrid fp16/fp8 PE version.

Computes y[b, o] = sum_k exp(log_weight[o, k]) * x[b, o*K + k]
for x [16384, 8192] fp32, log_weight [1024, 8] fp32.

Data-parallel over batch across 8 NeuronCores (2048 rows each).  Builds
on the fp16 PE kernel (112us), cutting DMA bytes further with weight-
aware mixed precision: for each output o, the 2 features with the
largest w = exp(log_weight) stay fp16 ("hot", N_HOT per output), the
rest go fp8 e4m3 ("cold").  Quantization error is dominated by
w-amplified terms, so routing only the top weight per output through
fp16 gives rel err 1.24e-2 (deterministic on the fixed-seed inputs,
reproduced to 7 digits on HW; tolerance 2e-2; N_HOT=2 gives 6.3e-3 at
+5us if more margin is ever needed).

The host permutes features into per-output-sorted order, which makes
EVERY stationary diagonal: chunk c = 8t + j of group t holds feature
rank j of output o = 128t + p at row p, so S_c = diag(w-rank-j).  The
device builds all 64 [128,128] stationary blocks from a 16KB weight
table with one tensor_scalar_mul each against an uploaded identity
(per-partition scalar broadcast; fp16 out for hot chunks, fp8 for cold).

Per group and batch-quarter b: 2 hot fp16 matmuls accumulate in a hot
PSUM bank, 6 cold fp8 matmuls in a cold bank (groups never mix dtypes
in one accumulation bank — mixed groups wedge the device, as do
mixed-K partial accumulations).  The hot bank evacuates early via ACT
copy to an fp32 staging tile (freeing hot banks mid-group), then DVE
fuses (cold + staged hot) -> fp16 with one scalar_tensor_tensor per
bank.  Stores batch per group into one [128, 2048] DMA (4KB
descriptors; 1KB-descriptor stores are desc-gen-bound at ~6ns/desc).

Streams: hot x [1024, 2048] f16 + cold x [7168, 2048] f8 + y out
[1024, 2048] f16 = 23.1 MB/core vs 37.8 for pure fp16.  x loads ride
Sync HWDGE; tables + y stores ride ScalarE HWDGE.  HWDGE queues cannot
start before ~8.7us (NEFF preamble).  PE: each stationary self-loads
once (b=0) and the 3 reuse matmuls set ins.ldweights=False — skipping
192 redundant 128-cycle reloads (~8us measured; the public matmul API
has no reuse path and walrus --enable-ldw-opt is hardcoded off).
Measured 85.8-86.0us (baseline 212.5us; pure-fp16 PE version 112us).
"""

import numpy as np

B = 16384
IN_F = 8192
OUT_F = 1024
K = 8
N_CORES = 8
P = 128
N_HOT = 1  # hot (fp16) feature ranks per output; rest are fp8

_CACHE = {}


def _build(b_shard, in_f, out_f, n_cores, x_bufs=6, c_bufs=18, y_bufs=3, nb=4):
    """Build + compile the per-core Bass module (SPMD across n_cores)."""
    from concourse import bacc, tile, mybir

    f16 = mybir.dt.float16
    f8 = mybir.dt.float8e4
    f32 = mybir.dt.float32
    n_groups = out_f // P  # 8 output groups of 128
    n_cold = 8 - N_HOT
    bw = b_shard // nb  # batch columns per PSUM bank (512 fp32 = 1 bank)

    nc = bacc.Bacc(
        "TRN2",
        target_bir_lowering=False,
        debug=False,
        enable_asserts=True,
        num_devices=n_cores,
    )
    xh_d = nc.dram_tensor("xh", [n_groups * N_HOT * P, b_shard], f16, kind="ExternalInput")
    xc_d = nc.dram_tensor("xc", [n_groups * n_cold * P, b_shard], f8, kind="ExternalInput")
    v_d = nc.dram_tensor("v", [P, 8 * n_groups], f32, kind="ExternalInput")
    id_d = nc.dram_tensor("idm", [P, P], f16, kind="ExternalInput")
    y_d = nc.dram_tensor("y", [out_f, b_shard], f16, kind="ExternalOutput")

    with tile.TileContext(nc) as tc:
        with (
            tc.tile_pool(name="consts", bufs=1) as cpool,
            tc.tile_pool(name="xh", bufs=x_bufs) as hpool,
            tc.tile_pool(name="xc", bufs=c_bufs) as cxpool,
            tc.tile_pool(name="ys", bufs=y_bufs) as ypool,
            tc.tile_pool(name="yh", bufs=2) as spool,
            tc.tile_pool(name="ps", bufs=4, space="PSUM") as ppool,
        ):
            ident = cpool.tile([P, P], f16, tag="id")
            vt = cpool.tile([P, 8 * n_groups], f32, tag="v")
            st16 = cpool.tile([P, n_groups * N_HOT * P], f16, tag="s16")
            st8 = cpool.tile([P, n_groups * n_cold * P], f8, tag="s8")
            nc.scalar.dma_start(out=ident[:], in_=id_d[:])
            nc.scalar.dma_start(out=vt[:], in_=v_d[:])
            for t in range(n_groups):
                last = t == n_groups - 1
                # Group t's diagonal stationaries: S = ident * v[:, c]
                # broadcast per partition (v[p, 8t+j] = rank-j weight of
                # output 128t+p).  Emitted INSIDE the loop so the DVE FIFO
                # interleaves builds with the PSUM-freeing fuse ops — built
                # up front, every fuse queues behind ~15us of builds and the
                # PE stalls on PSUM recycling.
                from concourse import mybir as mb

                id_h = ident[:].rearrange("p (x m) -> p x m", x=1).broadcast_to([P, N_HOT, P])
                v_h = (
                    vt[:, 8 * t : 8 * t + N_HOT]
                    .rearrange("p (j x) -> p j x", x=1)
                    .broadcast_to([P, N_HOT, P])
                )
                nc.vector.scalar_tensor_tensor(
                    st16[:, t * N_HOT * P : (t + 1) * N_HOT * P].rearrange(
                        "p (j m) -> p j m", j=N_HOT
                    ),
                    id_h, 1.0, v_h, mb.AluOpType.mult, mb.AluOpType.mult,
                )
                id_c = ident[:].rearrange("p (x m) -> p x m", x=1).broadcast_to([P, n_cold, P])
                v_c = (
                    vt[:, 8 * t + N_HOT : 8 * t + 8]
                    .rearrange("p (j x) -> p j x", x=1)
                    .broadcast_to([P, n_cold, P])
                )
                nc.vector.scalar_tensor_tensor(
                    st8[:, t * n_cold * P : (t + 1) * n_cold * P].rearrange(
                        "p (j m) -> p j m", j=n_cold
                    ),
                    id_c, 1.0, v_c, mb.AluOpType.mult, mb.AluOpType.mult,
                )
                hbs = [
                    ppool.tile([P, bw], f32, tag="ph", name=f"ph_{t}_{b}")
                    for b in range(nb)
                ]
                cbs = [
                    ppool.tile([P, bw], f32, tag="pc", name=f"pc_{t}_{b}")
                    for b in range(nb)
                ]
                # loads in consumption order: hot ranks 0..1, then cold 0..5
                hts, cts = [], []
                for j in range(N_HOT):
                    r = (t * N_HOT + j) * P
                    xt = hpool.tile([P, b_shard], f16, tag="xh", name=f"xh_{t}_{j}")
                    nc.sync.dma_start(out=xt[:], in_=xh_d[r : r + P, :])
                    hts.append(xt)
                for j in range(n_cold):
                    r = (t * n_cold + j) * P
                    xt = cxpool.tile([P, b_shard], f8, tag="xc", name=f"xc_{t}_{j}")
                    nc.sync.dma_start(out=xt[:], in_=xc_d[r : r + P, :])
                    cts.append(xt)
                # hot accumulation (fp16 x fp16), 2 matmuls per bank
                for j in range(N_HOT):
                    hc = t * N_HOT + j
                    for b in range(nb):
                        mm = nc.tensor.matmul(
                            hbs[b][:],
                            lhsT=st16[:, hc * P : (hc + 1) * P],
                            rhs=hts[j][:, b * bw : (b + 1) * bw],
                            start=(j == 0),
                            stop=(j == N_HOT - 1),
                        )
                        if b > 0:
                            # stationary already in the PE array from b=0:
                            # skip the redundant 128-cycle weight reload
                            mm.ins.ldweights = False
                # evacuate hot banks early to fp32 staging (frees them for
                # the next group while cold matmuls still run)
                ysb = spool.tile([P, b_shard], f32, tag="yh", name=f"yh_{t}")
                for b in range(nb):
                    nc.scalar.copy(out=ysb[:, b * bw : (b + 1) * bw], in_=hbs[b][:])
                # cold accumulation (fp8 x fp8), 6 matmuls per bank
                for j in range(n_cold):
                    cc = t * n_cold + j
                    for b in range(nb):
                        mm = nc.tensor.matmul(
                            cbs[b][:],
                            lhsT=st8[:, cc * P : (cc + 1) * P],
                            rhs=cts[j][:, b * bw : (b + 1) * bw],
                            start=(j == 0),
                            stop=(j == n_cold - 1),
                        )
                        if b > 0:
                            mm.ins.ldweights = False
                yt = ypool.tile([P, b_shard], f16, tag="y", name=f"y_{t}")
                if last:
                    # tail: fuse straight from PSUM per bank (no staging
                    # copy), store each half as soon as its banks fuse
                    for b in range(nb):
                        nc.vector.scalar_tensor_tensor(
                            yt[:, b * bw : (b + 1) * bw],
                            cbs[b][:],
                            1.0,
                            ysb[:, b * bw : (b + 1) * bw],
                            mb.AluOpType.mult,
                            mb.AluOpType.add,
                        )
                        if b == 1:
                            nc.scalar.dma_start(
                                out=y_d[t * P : (t + 1) * P, 0 : 2 * bw],
                                in_=yt[:, 0 : 2 * bw],
                            )
                    nc.scalar.dma_start(
                        out=y_d[t * P : (t + 1) * P, 2 * bw :], in_=yt[:, 2 * bw :]
                    )
                else:
                    # free cold banks fast via ACT copies to fp32 staging;
                    # fuse cold + hot -> fp16 on DVE off the PSUM critical path
                    csb = spool.tile([P, b_shard], f32, tag="yc", name=f"yc_{t}")
                    for b in range(nb):
                        nc.scalar.copy(out=csb[:, b * bw : (b + 1) * bw], in_=cbs[b][:])
                    for b in range(nb):
                        nc.vector.scalar_tensor_tensor(
                            yt[:, b * bw : (b + 1) * bw],
                            csb[:, b * bw : (b + 1) * bw],
                            1.0,
                            ysb[:, b * bw : (b + 1) * bw],
                            mb.AluOpType.mult,
                            mb.AluOpType.add,
                        )
                    nc.scalar.dma_start(out=y_d[t * P : (t + 1) * P, :], in_=yt[:])
    nc.compile()
    return nc


def _prep_inputs(x, log_weight, b_shard):
    """Host-side: per-output weight sort, permutation gather, mixed casts."""
    from concourse import mybir

    f8np = mybir.dt.np(mybir.dt.float8e4)
    w = np.exp(np.asarray(log_weight, np.float64)).astype(np.float32)  # [1024, 8]
    ordk = np.argsort(-w, axis=1)  # [1024, 8] feature ranks per output
    o_all = np.arange(OUT_F)
    wsort = w[o_all[:, None], ordk]  # [1024, 8]
    # v[p, 8t+j] = wsort[128t+p, j]
    v = np.ascontiguousarray(
        wsort.reshape(8, P, 8).transpose(1, 0, 2).reshape(P, 64), dtype=np.float32
    )
    feat = (8 * o_all[:, None] + ordk).reshape(8, P, 8)  # [t, p, j]
    hot_idx = feat[:, :, :N_HOT].transpose(0, 2, 1).reshape(-1)  # [(t*2+j)*128+p]
    cold_idx = feat[:, :, N_HOT:].transpose(0, 2, 1).reshape(-1)
    ident = np.eye(P, dtype=np.float16)

    x16 = np.asarray(x, np.float32).astype(np.float16)
    in_maps = []
    for i in range(N_CORES):
        xt = np.ascontiguousarray(x16[i * b_shard : (i + 1) * b_shard].T)
        xh = np.ascontiguousarray(xt[hot_idx])
        xc = np.ascontiguousarray(xt[cold_idx]).astype(f8np)
        in_maps.append({"xh": xh, "xc": xc, "v": v, "idm": ident})
    return in_maps


def kernel(x, log_weight, _trace_dir=None):
    from concourse import bass_utils

    b_shard = B // N_CORES
    if "nc" not in _CACHE:
        _CACHE["nc"] = _build(b_shard, IN_F, OUT_F, N_CORES)
    nc = _CACHE["nc"]

    in_maps = _prep_inputs(x, log_weight, b_shard)
    kwargs = {}
    if _trace_dir is not None:
        kwargs = {"trace": True, "tmpdir": _trace_dir}
    res = bass_utils.run_bass_kernel_spmd(
        nc, in_maps, core_ids=list(range(N_CORES)), **kwargs
    )
    _CACHE["last_res"] = res
    y = np.empty((B, OUT_F), np.float32)
    for i in range(N_CORES):
        y[i * b_shard : (i + 1) * b_shard] = res.results[i]["y"].T.astype(np.float32)
    return y
